# revision 6
# baseline (speedup 1.0000x reference)
"""Bass/Trainium2 kernel for nn_LogReg_8151847928094 — scatter-table design.

out[b] = sum_s w[text[s, b]] + bias   (bag-of-words logistic regression)

Design (8 NeuronCores, batch-sharded 2048 -> 8 x 256 columns):
  Token t = hi*128 + p lives at partition p = t%128 of the compact bf16
  table wr[p, hi] = w[hi*128+p] ([128, 784], hi < 782). Per core the
  ~51200 tokens average 0.5 per (p, hi) cell, so instead of gathering
  per token, the TABLE ITSELF is streamed Q times through
  gpsimd.local_scatter with per-partition index maps:

      slotmap[p, q*784 + hi] = column-slot of the q-th token at (p, hi)
      dst[p, slot] = wr[p, hi]        (slot = b*7 + r, r < 7)

  local_scatter reads both streams sequentially (no per-index SBUF
  commands), scatters in Q7-local RAM, and writes dst back densely —
  ~5 cyc per 2 stream elements. Tokens ranked >= 7 within their (p, b)
  cell go through a second small local_scatter into dst2 (slot b*2+r').
  DVE reduces dst over the run dim, PE ones-matmul reduces partitions,
  add bias, DMA out. Only one GPSIMD library (local_scatter) is used,
  so no per-iteration library reloads.
"""

import sys

sys.path.insert(0, "/opt/trn_rl_repo")

import numpy as np
import ml_dtypes

import concourse.bass as bass
import concourse.bacc as bacc
import concourse.mybir as mybir
import concourse.tile as tile
from concourse.bass_utils import run_bass_kernel_spmd

S = 200
B = 2048
V = 100000
NCORES = 8
BS = B // NCORES  # 256
P = 128
HI = (V + P - 1) // P  # 782
HIP = HI + 2  # 784, even, last two columns are zero pad
R3A = 7  # dst1 slots per (p, b): 256*7 = 1792 <= 2046
R3B = 6  # dst2 slots per (p, b): 256*6 = 1536 (ranks 7..12)

_prog_cache = {}


def _rank_within(key):
    """rank of each element within its equal-key group (stable)."""
    order = np.argsort(key, kind="stable")
    ks = key[order]
    newgrp = np.r_[True, ks[1:] != ks[:-1]]
    starts = np.flatnonzero(newgrp)
    run_ids = np.cumsum(newgrp) - 1
    rank_sorted = np.arange(ks.size) - starts[run_ids]
    rank = np.empty(ks.size, np.int64)
    rank[order] = rank_sorted
    return rank


def _pack_core(tokens):
    """tokens [S, BS] -> (sm1 [128, Q1*HIP], sm2 [128, Q2*HIP], Q1, Q2)

    Call A takes at most 2 copies of each (p, hi) value (Q1 = 2 by
    construction) and at most 7 tokens per (p, b) cell; everything else
    (value copies >= 2 and rank overflow) rides call B, whose rounds
    absorb both, hitting the max-multiplicity round floor exactly."""
    t = np.asarray(tokens, np.int64).ravel(order="F")
    b = np.repeat(np.arange(BS, dtype=np.int64), S)
    hi = t // P
    p = (t + hi) % P  # skewed layout: decorrelates mod-128 clustering

    qv = _rank_within(p * HIP + hi)  # same-value copy rank within the core
    eligA = qv < 2
    r3a = np.full(t.size, 1 << 30, np.int64)
    r3a[eligA] = _rank_within((p * BS + b)[eligA])
    inA = eligA & (r3a < R3A)
    inB = ~inA
    r3b = _rank_within((p * BS + b)[inB])
    assert r3b.max() < R3B if r3b.size else True, f"B overflow: {r3b.max()}"

    # call A: data round = qv (< 2), slot = b*7 + r3a
    q1 = int(qv[inA].max()) + 1 if inA.any() else 1
    sm1 = np.full((P, q1 * HIP), -1, np.int16)
    sm1[p[inA], qv[inA] * HIP + hi[inA]] = (b * R3A + r3a)[inA]

    # call B: own value-copy rounds, slot = b*6 + r3b
    qB = _rank_within((p * HIP + hi)[inB])
    q2 = int(qB.max()) + 1 if inB.any() else 1
    sm2 = np.full((P, q2 * HIP), -1, np.int16)
    sm2[p[inB], qB * HIP + hi[inB]] = b[inB] * R3B + r3b
    return sm1, sm2, q1, q2


def _build_program(Q1, Q2, QW, loop_T=None):
    NW = QW * HIP
    nc = bacc.Bacc("TRN2", target_bir_lowering=False, debug=False)
    wrq_d = nc.declare_dram_parameter("wrq", [P, NW], mybir.dt.bfloat16, isOutput=False)
    sm1_d = nc.declare_dram_parameter("sm1", [P, Q1 * HIP], mybir.dt.int16, isOutput=False)
    sm2_d = nc.declare_dram_parameter("sm2", [P, Q2 * HIP], mybir.dt.int16, isOutput=False)
    bias_d = nc.declare_dram_parameter("bias", [1, BS], mybir.dt.float32, isOutput=False)
    out_d = nc.declare_dram_parameter("out", [1, BS], mybir.dt.float32, isOutput=True)

    with tile.TileContext(nc) as tc:
        with (
            tc.tile_pool(name="sbuf", bufs=1) as pool,
            tc.tile_pool(name="dbuf", bufs=2) as dpool,
            tc.tile_pool(name="psum", bufs=1, space="PSUM") as psum_pool,
        ):
            wrq_t = dpool.tile([P, NW], mybir.dt.bfloat16)
            sm1_t = dpool.tile([P, Q1 * HIP], mybir.dt.int16)
            sm2_t = dpool.tile([P, Q2 * HIP], mybir.dt.int16)
            dst1_t = pool.tile([P, BS * R3A], mybir.dt.bfloat16)
            dst2_t = pool.tile([P, BS * R3B], mybir.dt.bfloat16)
            red1_t = pool.tile([P, BS], mybir.dt.float32)
            red2_t = pool.tile([P, BS], mybir.dt.float32)
            sum_t = pool.tile([P, BS], mybir.dt.float32)
            ones_t = pool.tile([P, 1], mybir.dt.float32)
            bias_t = pool.tile([1, BS], mybir.dt.float32)
            res_t = pool.tile([1, BS], mybir.dt.float32)
            psum_t = psum_pool.tile([1, BS], mybir.dt.float32)

            nc.vector.memset(ones_t[:], 1.0)

            def body():
                nc.sync.dma_start(out=wrq_t[:], in_=wrq_d[:])
                nc.sync.dma_start(out=sm1_t[:], in_=sm1_d[:])
                nc.sync.dma_start(out=sm2_t[:], in_=sm2_d[:])
                nc.sync.dma_start(out=bias_t[:], in_=bias_d[:])
                nc.gpsimd.local_scatter(
                    dst1_t[:],
                    wrq_t[:, : Q1 * HIP],
                    sm1_t[:],
                    channels=P,
                    num_elems=BS * R3A,
                    num_idxs=Q1 * HIP,
                )
                nc.gpsimd.local_scatter(
                    dst2_t[:],
                    wrq_t[:, : Q2 * HIP],
                    sm2_t[:],
                    channels=P,
                    num_elems=BS * R3B,
                    num_idxs=Q2 * HIP,
                )
                nc.vector.tensor_reduce(
                    out=red1_t[:],
                    in_=dst1_t[:].rearrange("p (b r) -> p b r", r=R3A),
                    axis=mybir.AxisListType.X,
                    op=mybir.AluOpType.add,
                )
                nc.vector.tensor_reduce(
                    out=red2_t[:],
                    in_=dst2_t[:].rearrange("p (b r) -> p b r", r=R3B),
                    axis=mybir.AxisListType.X,
                    op=mybir.AluOpType.add,
                )
                nc.vector.tensor_tensor(
                    out=sum_t[:], in0=red1_t[:], in1=red2_t[:], op=mybir.AluOpType.add
                )
                nc.tensor.matmul(
                    psum_t[:], lhsT=ones_t[:], rhs=sum_t[:], start=True, stop=True
                )
                nc.vector.tensor_tensor(
                    out=res_t[:], in0=psum_t[:], in1=bias_t[:], op=mybir.AluOpType.add
                )
                nc.sync.dma_start(out=out_d[:], in_=res_t[:])

            if loop_T is None:
                body()
            else:
                with tc.For_i(0, loop_T, 1) as _i:
                    body()
    nc.compile()
    return nc


def _shard_columns(text):
    """Assign the 2048 columns to 8 cores (256 each), minimizing per-core
    same-value multiplicity (which sets the local_scatter round count Q1).
    Greedy hottest-column-first. Returns perm [NCORES, BS] of column ids."""
    t64 = np.asarray(text, np.int64)
    vc = np.bincount(t64.ravel(), minlength=V)
    hot = vc[t64].max(axis=0)
    order = np.argsort(-hot, kind="stable")
    counts = np.zeros((NCORES, V), np.int16)
    load = np.zeros(NCORES, np.int64)
    assign = np.full(B, -1, np.int32)
    for j in order:
        vals = t64[:, j]
        proj = counts[:, vals].max(axis=1).astype(np.int64)
        score = (proj >= 3) * np.int64(1 << 20) + proj * (B * 8) + load
        score[load >= BS] = np.int64(1) << 40
        c = int(score.argmin())
        assign[j] = c
        np.add.at(counts[c], vals, 1)
        load[c] += 1
    perm = np.stack([np.flatnonzero(assign == c) for c in range(NCORES)])
    assert perm.shape == (NCORES, BS)
    assert np.array_equal(np.sort(perm.ravel()), np.arange(B))
    return perm


def _host_pack(text, w, b):
    """Full host-side prep shared by kernel() and the bench harness."""
    text = np.asarray(text)
    w = np.asarray(w, dtype=np.float32).reshape(-1)
    b = np.asarray(b, dtype=np.float32).reshape(-1)

    perm = _shard_columns(text)
    packs = [_pack_core(text[:, perm[c]]) for c in range(NCORES)]
    Q1 = max(pk[2] for pk in packs)
    Q2 = max(pk[3] for pk in packs)
    QW = max(Q1, Q2)

    w_pad = np.zeros(HIP * P, np.float32)
    w_pad[:V] = w
    # skewed table: wr[p, hi] = w[hi*128 + ((p - hi) % 128)]
    hi_g, p_g = np.meshgrid(np.arange(HIP), np.arange(P), indexing="xy")
    wr = w_pad.reshape(HIP, P)[hi_g, (p_g - hi_g) % P].astype(ml_dtypes.bfloat16)
    assert wr.shape == (P, HIP)
    wrq = np.tile(wr, (1, QW))  # [128, QW*HIP]
    bias_row = np.full((1, BS), b[0], np.float32)

    in_maps = []
    for c in range(NCORES):
        sm1, sm2, q1, q2 = packs[c]
        if q1 < Q1:
            sm1 = np.concatenate(
                [sm1, np.full((P, (Q1 - q1) * HIP), -1, np.int16)], axis=1
            )
        if q2 < Q2:
            sm2 = np.concatenate(
                [sm2, np.full((P, (Q2 - q2) * HIP), -1, np.int16)], axis=1
            )
        in_maps.append({"wrq": wrq, "sm1": sm1, "sm2": sm2, "bias": bias_row})
    return in_maps, Q1, Q2, QW, perm


def kernel(text, w, b):
    in_maps, Q1, Q2, QW, perm = _host_pack(text, w, b)
    nc = _prog_cache.get((Q1, Q2))
    if nc is None:
        nc = _build_program(Q1, Q2, QW)
        _prog_cache[(Q1, Q2)] = nc
    res = run_bass_kernel_spmd(nc, in_maps, list(range(NCORES))).results
    out = np.empty(B, np.float32)
    for c in range(NCORES):
        out[perm[c]] = res[c]["out"][0]
    return out.astype(np.float32)


if __name__ == "__main__":
    rng = np.random.default_rng(0)
    text = rng.integers(0, V, (S, B)).astype(np.int64)
    w = (rng.standard_normal((1, V)) * 0.01).astype(np.float32)
    b = np.zeros((1,), np.float32)
    out = kernel(text, w, b)
    exp = w[0][text].sum(axis=0) + b[0]
    err = np.abs(out - exp).max() / (np.abs(exp).max() + 1e-9)
    print("rel err:", err)
